# revision 2
# baseline (speedup 1.0000x reference)
"""GCNConv (N=20000, E=320000, D=1024) on 8 trn2 NeuronCores.

out = segment_sum(norm * h[col] -> row),  h = x @ W^T + b,
norm = deg^-1/2[row] * deg^-1/2[col], with self-loops added.

Sharding: nodes split 2500/core (padded to 2560 = 20 blocks of 128).
Per core: h_scaled = (dis*x) @ W^T + dis (x) b  (fp32 matmul, bf16 store),
AllGather h_scaled, then edges partitioned by destination: per 128-dest
block, gather source rows (indirect DMA), 0/1 selection matrix via
iota==dest_local, segment-sum via PE matmul accumulate in fp32 PSUM,
scaled by dis[dest] on copy-out.
"""

import numpy as np

import concourse.bacc as bacc
import concourse.mybir as mybir
import concourse.tile as tile
from concourse import bass
from concourse import bass_utils

N = 20000
E = 320000
D = 1024
NC = 8
NPC = N // NC            # 2500 real nodes per core
NBLK = 20                # dest blocks of 128 per core
NPCP = NBLK * 128        # 2560 padded nodes per core
P = 128
KT = D // P              # 8 contraction tiles

_cache = {}


def _preprocess(x, edge_index, W, b):
    x = np.asarray(x, dtype=np.float32)
    ei = np.asarray(edge_index)
    W = np.asarray(W, dtype=np.float32)
    b = np.asarray(b, dtype=np.float32)

    self_idx = np.arange(N, dtype=np.int64)
    row = np.concatenate([ei[0].astype(np.int64), self_idx])
    col = np.concatenate([ei[1].astype(np.int64), self_idx])

    deg = np.bincount(row, minlength=N).astype(np.float32)
    dis = deg ** -0.5  # deg >= 1 (self loops)

    # padded-global index of each source column in the gathered table
    colp = ((col // NPC) * NPCP + (col % NPC)).astype(np.int32)

    core_of = (row // NPC).astype(np.int32)
    rl = (row - core_of.astype(np.int64) * NPC).astype(np.int32)  # local dest

    # per (core, block) edge lists
    per_cb_cols = [[None] * NBLK for _ in range(NC)]
    per_cb_dl = [[None] * NBLK for _ in range(NC)]
    for c in range(NC):
        m = core_of == c
        rl_c = rl[m]
        cp_c = colp[m]
        order = np.argsort(rl_c, kind="stable")
        rl_c = rl_c[order]
        cp_c = cp_c[order]
        blk = rl_c // P
        bounds = np.searchsorted(blk, np.arange(NBLK + 1))
        for bk in range(NBLK):
            s, e = bounds[bk], bounds[bk + 1]
            per_cb_cols[c][bk] = cp_c[s:e]
            per_cb_dl[c][bk] = rl_c[s:e] - bk * P

    # shared tile schedule: T_b = max over cores of ceil(edges/128)
    T_bs = []
    for bk in range(NBLK):
        mx = max(len(per_cb_cols[c][bk]) for c in range(NC))
        T_bs.append(max(1, -(-mx // P)))
    NT = sum(T_bs)

    # per-core padded [NT, 128] index / dest_local arrays
    colidx = np.zeros((NC, NT, P), dtype=np.int32)
    dl = np.full((NC, NT, P), -1.0, dtype=np.float32)
    t0 = 0
    for bk in range(NBLK):
        for c in range(NC):
            cc = per_cb_cols[c][bk]
            dd = per_cb_dl[c][bk]
            n = len(cc)
            flat_c = colidx[c, t0 : t0 + T_bs[bk]].reshape(-1)
            flat_d = dl[c, t0 : t0 + T_bs[bk]].reshape(-1)
            flat_c[:n] = cc
            flat_d[:n] = dd.astype(np.float32)
        t0 += T_bs[bk]

    WT = np.ascontiguousarray(W.T)  # [d, o]
    bvec = b.reshape(1, D)

    in_maps = []
    for c in range(NC):
        rows = slice(c * NPC, (c + 1) * NPC)
        dis_c = dis[rows]
        xs = x[rows] * dis_c[:, None]
        xT = np.zeros((D, NPCP), dtype=np.float32)
        xT[:, :NPC] = xs.T
        disv = np.zeros((1, NPCP), dtype=np.float32)
        disv[0, :NPC] = dis_c
        disT = np.ascontiguousarray(
            disv.reshape(NBLK, P).T
        )  # [128, NBLK]
        in_maps.append(
            {
                "xT": xT,
                "WT": WT,
                "bvec": bvec,
                "disv": disv,
                "disT": disT,
                "colidx": np.ascontiguousarray(colidx[c].T),  # [128, NT]
                "dl": np.ascontiguousarray(dl[c].T),  # [128, NT]
            }
        )
    return tuple(T_bs), NT, in_maps


def _build(T_bs, NT):
    f32 = mybir.dt.float32
    bf16 = mybir.dt.bfloat16
    i32 = mybir.dt.int32

    nc = bacc.Bacc("TRN2", target_bir_lowering=False, debug=False, num_devices=NC)
    xT = nc.dram_tensor("xT", [D, NPCP], f32, kind="ExternalInput").ap()
    WT = nc.dram_tensor("WT", [D, D], f32, kind="ExternalInput").ap()
    bvec = nc.dram_tensor("bvec", [1, D], f32, kind="ExternalInput").ap()
    disv = nc.dram_tensor("disv", [1, NPCP], f32, kind="ExternalInput").ap()
    disT = nc.dram_tensor("disT", [P, NBLK], f32, kind="ExternalInput").ap()
    colidx = nc.dram_tensor("colidx", [P, NT], i32, kind="ExternalInput").ap()
    dl = nc.dram_tensor("dl", [P, NT], f32, kind="ExternalInput").ap()
    yout = nc.dram_tensor("yout", [NPCP, D], f32, kind="ExternalOutput").ap()

    with tile.TileContext(nc) as tc:
        with tc.tile_pool(name="dram", bufs=1, space="DRAM") as dram, \
             tc.tile_pool(name="const", bufs=1) as const:
            h_local = dram.tile([NPCP, D], bf16)
            hg = dram.tile([NC * NPCP, D], bf16, addr_space="Shared")

            # constants in SBUF
            wt_sb = const.tile([P, KT * D], f32, name="wt_sb")
            for k in range(KT):
                nc.sync.dma_start(
                    wt_sb[:, k * D : (k + 1) * D], WT[k * P : (k + 1) * P, :]
                )
            bv_sb = const.tile([1, D], f32, name="bv_sb")
            nc.sync.dma_start(bv_sb[:], bvec[:])
            disv_sb = const.tile([1, NPCP], f32, name="disv_sb")
            nc.sync.dma_start(disv_sb[:], disv[:])
            disT_sb = const.tile([P, NBLK], f32, name="disT_sb")
            nc.sync.dma_start(disT_sb[:], disT[:])
            ci_sb = const.tile([P, NT], i32, name="ci_sb")
            nc.sync.dma_start(ci_sb[:], colidx[:])
            dl_sb = const.tile([P, NT], f32, name="dl_sb")
            nc.sync.dma_start(dl_sb[:], dl[:])
            iota_sb = const.tile([P, P], f32, name="iota_sb")
            iota_i = const.tile([P, P], i32, name="iota_i")
            nc.gpsimd.iota(iota_i[:], pattern=[[1, P]], channel_multiplier=0)
            nc.vector.tensor_copy(iota_sb[:], iota_i[:])

            # ---------------- h phase ----------------
            with tc.tile_pool(name="xk", bufs=1) as xkp, \
                 tc.tile_pool(name="hps", bufs=2, space="PSUM") as hps, \
                 tc.tile_pool(name="hout", bufs=3) as houtp:
                xk_sb = xkp.tile([P, KT * NPCP], f32, name="xk_sb")
                for k in range(KT):
                    nc.sync.dma_start(
                        xk_sb[:, k * NPCP : (k + 1) * NPCP],
                        xT[k * P : (k + 1) * P, :],
                    )
                chunks = [slice(s, min(s + 512, D)) for s in range(0, D, 512)]
                for j in range(NBLK):
                    ps = hps.tile([P, D], f32)
                    for cs in chunks:
                        nc.tensor.matmul(
                            ps[:, cs],
                            lhsT=disv_sb[:, j * P : (j + 1) * P],
                            rhs=bv_sb[:, cs],
                            start=True,
                            stop=False,
                        )
                    for k in range(KT):
                        lhsT = xk_sb[:, k * NPCP + j * P : k * NPCP + (j + 1) * P]
                        for cs in chunks:
                            nc.tensor.matmul(
                                ps[:, cs],
                                lhsT=lhsT,
                                rhs=wt_sb[:, k * D + cs.start : k * D + cs.stop],
                                start=False,
                                stop=(k == KT - 1),
                            )
                    hsb = houtp.tile([P, D], bf16)
                    nc.vector.tensor_copy(hsb[:], ps[:])
                    nc.sync.dma_start(h_local[j * P : (j + 1) * P, :], hsb[:])

            # ---------------- AllGather ----------------
            nc.gpsimd.collective_compute(
                "AllGather",
                mybir.AluOpType.bypass,
                replica_groups=[list(range(NC))],
                ins=[h_local[:]],
                outs=[hg[:]],
            )

            # ---------------- aggregation phase ----------------
            with tc.tile_pool(name="gath", bufs=6) as gp, \
                 tc.tile_pool(name="sel", bufs=6) as selp, \
                 tc.tile_pool(name="aps", bufs=2, space="PSUM") as aps, \
                 tc.tile_pool(name="aout", bufs=3) as aoutp:
                t = 0
                for bk in range(NBLK):
                    ps = aps.tile([P, D], f32)
                    Tb = T_bs[bk]
                    for i in range(Tb):
                        g = gp.tile([P, D], bf16)
                        nc.gpsimd.indirect_dma_start(
                            out=g[:],
                            out_offset=None,
                            in_=hg[:],
                            in_offset=bass.IndirectOffsetOnAxis(
                                ap=ci_sb[:, t : t + 1], axis=0
                            ),
                        )
                        sel = selp.tile([P, P], bf16)
                        nc.vector.tensor_scalar(
                            out=sel[:],
                            in0=iota_sb[:],
                            scalar1=dl_sb[:, t : t + 1],
                            scalar2=None,
                            op0=mybir.AluOpType.is_equal,
                        )
                        for cs in [slice(s, min(s + 512, D)) for s in range(0, D, 512)]:
                            nc.tensor.matmul(
                                ps[:, cs],
                                lhsT=sel[:],
                                rhs=g[:, cs],
                                start=(i == 0),
                                stop=(i == Tb - 1),
                            )
                        t += 1
                    ob = aoutp.tile([P, D], f32)
                    nc.vector.tensor_scalar(
                        out=ob[:],
                        in0=ps[:],
                        scalar1=disT_sb[:, bk : bk + 1],
                        scalar2=None,
                        op0=mybir.AluOpType.mult,
                    )
                    nc.sync.dma_start(yout[bk * P : (bk + 1) * P, :], ob[:])

    nc.compile()
    return nc


def kernel(x, edge_index, W, b):
    T_bs, NT, in_maps = _preprocess(x, edge_index, W, b)
    key = (T_bs, NT)
    if key not in _cache:
        _cache[key] = _build(T_bs, NT)
    nc = _cache[key]
    res = bass_utils.run_bass_kernel_spmd(nc, in_maps, core_ids=list(range(NC)))
    out = np.empty((N, D), dtype=np.float32)
    for c in range(NC):
        out[c * NPC : (c + 1) * NPC] = res.results[c]["yout"][:NPC]
    return out


# revision 7
# speedup vs baseline: 1.1906x; 1.1906x over previous
"""GCNConv (N=20000, E=320000, D=1024) on 8 trn2 NeuronCores.

out = segment_sum(norm * h[col] -> row),  h = x @ W^T + b,
norm = deg^-1/2[row] * deg^-1/2[col], with self-loops added.

Sharding: nodes split 2500/core (padded to 2560 = 20 blocks of 128).
Per core: h_scaled = (dis*x) @ W^T + dis (x) b  (bf16 matmul + exact fp32
rank-1 bias, bf16 store), AllGather h_scaled in 4 chunks overlapped with
the h compute, then edges partitioned by destination: per 128-dest block,
bulk dma_gather of source rows, 0/1 selection matrices via
iota==dest_local, segment-sum via PE matmul accumulate in fp32 PSUM,
scaled by dis[dest] on copy-out.
"""

import numpy as np
import ml_dtypes

import concourse.bacc as bacc
import concourse.mybir as mybir
import concourse.tile as tile
from concourse import bass
from concourse import bass_utils

N = 20000
E = 320000
D = 1024
NC = 8
NPC = N // NC            # 2500 real nodes per core
NBLK = 20                # dest blocks of 128 per core
NPCP = NBLK * 128        # 2560 padded nodes per core
P = 128
KT = D // P              # 8 contraction tiles
CH = 1                   # AllGather chunks (each NBLK/CH dest blocks of h)

_cache = {}


def _preprocess(x, edge_index, W, b):
    x = np.asarray(x, dtype=np.float32)
    ei = np.asarray(edge_index)
    W = np.asarray(W, dtype=np.float32)
    b = np.asarray(b, dtype=np.float32)

    self_idx = np.arange(N, dtype=np.int64)
    row = np.concatenate([ei[0].astype(np.int64), self_idx])
    col = np.concatenate([ei[1].astype(np.int64), self_idx])

    deg = np.bincount(row, minlength=N).astype(np.float32)
    dis = deg ** -0.5  # deg >= 1 (self loops)

    # gathered-table index of each source column: the AllGather runs in CH
    # chunks of RCH rows per rank, so chunk ch of the table holds
    # [rank0 rows [ch*RCH,(ch+1)*RCH), rank1 rows ..., ...].
    CHe = min(CH, NBLK)
    RCH = NPCP // CHe
    csrc = (col // NPC).astype(np.int64)
    l = (col % NPC).astype(np.int64)
    chk = l // RCH
    colp = (chk * (NC * RCH) + csrc * RCH + (l - chk * RCH)).astype(np.int32)

    core_of = (row // NPC).astype(np.int32)
    rl = (row - core_of.astype(np.int64) * NPC).astype(np.int32)  # local dest

    per_cb_cols = [[None] * NBLK for _ in range(NC)]
    per_cb_dl = [[None] * NBLK for _ in range(NC)]
    for c in range(NC):
        m = core_of == c
        rl_c = rl[m]
        cp_c = colp[m]
        order = np.argsort(rl_c, kind="stable")
        rl_c = rl_c[order]
        cp_c = cp_c[order]
        blk = rl_c // P
        bounds = np.searchsorted(blk, np.arange(NBLK + 1))
        for bk in range(NBLK):
            s, e = bounds[bk], bounds[bk + 1]
            per_cb_cols[c][bk] = cp_c[s:e]
            per_cb_dl[c][bk] = rl_c[s:e] - bk * P

    # shared tile schedule: T_b = max over cores of ceil(edges/128)
    T_bs = []
    for bk in range(NBLK):
        mx = max(len(per_cb_cols[c][bk]) for c in range(NC))
        T_bs.append(max(1, -(-mx // P)))
    NT = sum(T_bs)

    # per-core padded arrays: dl [NT,128] f32; gather idx int16 packed
    # [16, T_b*8] per block (idx j at [j%16, j//16]), blocks concatenated
    # along columns, then replicated across the 8 Q7 16-partition stripes.
    dl = np.full((NC, NT, P), -1.0, dtype=np.float32)
    idx16 = np.zeros((NC, 16, NT * 8), dtype=np.int16)
    t0 = 0
    c0 = 0
    for bk in range(NBLK):
        ncols = T_bs[bk] * 8
        for c in range(NC):
            cc = per_cb_cols[c][bk]
            dd = per_cb_dl[c][bk]
            n = len(cc)
            flat_d = dl[c, t0 : t0 + T_bs[bk]].reshape(-1)
            flat_d[:n] = dd.astype(np.float32)
            buf = np.zeros(T_bs[bk] * P, dtype=np.int16)
            buf[:n] = cc.astype(np.int16)
            idx16[c, :, c0 : c0 + ncols] = buf.reshape(ncols, 16).T
        t0 += T_bs[bk]
        c0 += ncols

    WTb = np.ascontiguousarray(W.T).astype(ml_dtypes.bfloat16)  # [d, o]
    bvec = b.reshape(1, D)

    in_maps = []
    for c in range(NC):
        rows = slice(c * NPC, (c + 1) * NPC)
        dis_c = dis[rows]
        xs = x[rows] * dis_c[:, None]
        xT = np.zeros((D, NPCP), dtype=ml_dtypes.bfloat16)
        xT[:, :NPC] = xs.T.astype(ml_dtypes.bfloat16)
        disv = np.zeros((1, NPCP), dtype=np.float32)
        disv[0, :NPC] = dis_c
        disT = np.ascontiguousarray(disv.reshape(NBLK, P).T)  # [128, NBLK]
        in_maps.append(
            {
                "xT": xT,
                "WT": WTb,
                "bvec": bvec,
                "disv": disv,
                "disT": disT,
                "idx16": np.ascontiguousarray(np.tile(idx16[c], (8, 1))),
                "dl": np.ascontiguousarray(dl[c].T),  # [128, NT]
            }
        )
    return tuple(T_bs), NT, in_maps


def _build(T_bs, NT):
    f32 = mybir.dt.float32
    bf16 = mybir.dt.bfloat16
    i16 = mybir.dt.int16
    i32 = mybir.dt.int32
    CHe = min(CH, NBLK)
    RCH = NPCP // CHe
    JCH = NBLK // CHe  # h blocks per AG chunk

    nc = bacc.Bacc("TRN2", target_bir_lowering=False, debug=False, num_devices=NC)
    xT = nc.dram_tensor("xT", [D, NPCP], bf16, kind="ExternalInput").ap()
    WT = nc.dram_tensor("WT", [D, D], bf16, kind="ExternalInput").ap()
    bvec = nc.dram_tensor("bvec", [1, D], f32, kind="ExternalInput").ap()
    disv = nc.dram_tensor("disv", [1, NPCP], f32, kind="ExternalInput").ap()
    disT = nc.dram_tensor("disT", [P, NBLK], f32, kind="ExternalInput").ap()
    idx16 = nc.dram_tensor("idx16", [P, NT * 8], i16, kind="ExternalInput").ap()
    dl = nc.dram_tensor("dl", [P, NT], f32, kind="ExternalInput").ap()
    yout = nc.dram_tensor("yout", [NPCP, D], f32, kind="ExternalOutput").ap()

    with tile.TileContext(nc) as tc:
        with tc.tile_pool(name="dram", bufs=1, space="DRAM") as dram, \
             tc.tile_pool(name="const", bufs=1) as const:
            h_ch = [
                dram.tile([RCH, D], bf16, name=f"h_ch{ch}") for ch in range(CHe)
            ]
            hg = dram.tile([NC * NPCP, D], bf16)

            # constants in SBUF
            wt_sb = const.tile([P, KT * D], bf16, name="wt_sb")
            for k in range(KT):
                nc.sync.dma_start(
                    wt_sb[:, k * D : (k + 1) * D], WT[k * P : (k + 1) * P, :]
                )
            bv_sb = const.tile([1, D], f32, name="bv_sb")
            nc.sync.dma_start(bv_sb[:], bvec[:])
            disv_sb = const.tile([1, NPCP], f32, name="disv_sb")
            nc.sync.dma_start(disv_sb[:], disv[:])
            disT_sb = const.tile([P, NBLK], f32, name="disT_sb")
            nc.sync.dma_start(disT_sb[:], disT[:])
            ix_sb = const.tile([P, NT * 8], i16, name="ix_sb")
            nc.sync.dma_start(ix_sb[:], idx16[:])
            dl_sb = const.tile([P, NT], f32, name="dl_sb")
            nc.sync.dma_start(dl_sb[:], dl[:])
            iota_sb = const.tile([P, P], f32, name="iota_sb")
            iota_i = const.tile([P, P], i32, name="iota_i")
            nc.gpsimd.iota(iota_i[:], pattern=[[1, P]], channel_multiplier=0)
            nc.vector.tensor_copy(iota_sb[:], iota_i[:])

            # ---------------- h phase (+ chunked AllGather) ----------------
            with tc.tile_pool(name="xk", bufs=1) as xkp, \
                 tc.tile_pool(name="hps", bufs=2, space="PSUM") as hps, \
                 tc.tile_pool(name="hout", bufs=3) as houtp:
                xk_sb = xkp.tile([P, KT * NPCP], bf16, name="xk_sb")
                for k in range(KT):
                    nc.sync.dma_start(
                        xk_sb[:, k * NPCP : (k + 1) * NPCP],
                        xT[k * P : (k + 1) * P, :],
                    )
                chunks = [slice(s, min(s + 512, D)) for s in range(0, D, 512)]
                for j in range(NBLK):
                    ps = hps.tile([P, D], f32)
                    for cs in chunks:
                        nc.tensor.matmul(
                            ps[:, cs],
                            lhsT=disv_sb[:, j * P : (j + 1) * P],
                            rhs=bv_sb[:, cs],
                            start=True,
                            stop=False,
                        )
                    for k in range(KT):
                        lhsT = xk_sb[:, k * NPCP + j * P : k * NPCP + (j + 1) * P]
                        for cs in chunks:
                            nc.tensor.matmul(
                                ps[:, cs],
                                lhsT=lhsT,
                                rhs=wt_sb[:, k * D + cs.start : k * D + cs.stop],
                                start=False,
                                stop=(k == KT - 1),
                            )
                    hsb = houtp.tile([P, D], bf16)
                    nc.vector.tensor_copy(hsb[:], ps[:])
                    ch = j // JCH
                    jo = j - ch * JCH
                    nc.sync.dma_start(h_ch[ch][jo * P : (jo + 1) * P, :], hsb[:])
                    if jo == JCH - 1:
                        nc.gpsimd.collective_compute(
                            "AllGather",
                            mybir.AluOpType.bypass,
                            replica_groups=[list(range(NC))],
                            ins=[h_ch[ch][:]],
                            outs=[hg[ch * NC * RCH : (ch + 1) * NC * RCH, :]],
                        )

            # ---------------- aggregation phase ----------------
            with tc.tile_pool(name="gath", bufs=3) as gp, \
                 tc.tile_pool(name="sel", bufs=8) as selp, \
                 tc.tile_pool(name="aps", bufs=2, space="PSUM") as aps, \
                 tc.tile_pool(name="aout", bufs=3) as aoutp:
                t = 0
                c0 = 0
                for bk in range(NBLK):
                    Tb = T_bs[bk]
                    ni = Tb * P
                    g = gp.tile([P, Tb, D], bf16, tag="g")
                    nc.gpsimd.dma_gather(
                        g[:], hg[:], ix_sb[:, c0 : c0 + Tb * 8], ni, ni, D,
                        single_packet=False,
                    )
                    ps = aps.tile([P, D], f32)
                    for i in range(Tb):
                        sel = selp.tile([P, P], bf16)
                        nc.vector.tensor_scalar(
                            out=sel[:],
                            in0=iota_sb[:],
                            scalar1=dl_sb[:, t : t + 1],
                            scalar2=None,
                            op0=mybir.AluOpType.is_equal,
                        )
                        for cs in [slice(s, min(s + 512, D)) for s in range(0, D, 512)]:
                            nc.tensor.matmul(
                                ps[:, cs],
                                lhsT=sel[:],
                                rhs=g[:, i, cs],
                                start=(i == 0),
                                stop=(i == Tb - 1),
                            )
                        t += 1
                    ob = aoutp.tile([P, D], f32)
                    nc.vector.tensor_scalar(
                        out=ob[:],
                        in0=ps[:],
                        scalar1=disT_sb[:, bk : bk + 1],
                        scalar2=None,
                        op0=mybir.AluOpType.mult,
                    )
                    nc.sync.dma_start(yout[bk * P : (bk + 1) * P, :], ob[:])
                    c0 += Tb * 8

    nc.compile()
    return nc


def kernel(x, edge_index, W, b):
    T_bs, NT, in_maps = _preprocess(x, edge_index, W, b)
    key = (T_bs, NT)
    if key not in _cache:
        _cache[key] = _build(T_bs, NT)
    nc = _cache[key]
    res = bass_utils.run_bass_kernel_spmd(nc, in_maps, core_ids=list(range(NC)))
    out = np.empty((N, D), dtype=np.float32)
    for c in range(NC):
        out[c * NPC : (c + 1) * NPC] = res.results[c]["yout"][:NPC]
    return out


# revision 9
# speedup vs baseline: 1.4816x; 1.2443x over previous
"""GCNConv (N=20000, E=320000, D=1024) on 8 trn2 NeuronCores.

out = segment_sum(norm * h[col] -> row),  h = x @ W^T + b,
norm = deg^-1/2[row] * deg^-1/2[col], with self-loops added.

Sharding: nodes split 2500/core (padded to 2560 = 20 blocks of 128).
Per core: h_scaled = (dis*x) @ W^T + dis (x) b  (bf16 matmul + exact fp32
rank-1 bias, bf16 store), AllGather h_scaled in 4 chunks overlapped with
the h compute, then edges partitioned by destination: per 128-dest block,
bulk dma_gather of source rows, 0/1 selection matrices via
iota==dest_local, segment-sum via PE matmul accumulate in fp32 PSUM,
scaled by dis[dest] on copy-out.
"""

import numpy as np
import ml_dtypes

import concourse.bacc as bacc
import concourse.mybir as mybir
import concourse.tile as tile
from concourse import bass
from concourse import bass_utils

N = 20000
E = 320000
D = 1024
NC = 8
NPC = N // NC            # 2500 real nodes per core
NBLK = 20                # dest blocks of 128 per core
NPCP = NBLK * 128        # 2560 padded nodes per core
P = 128
KT = D // P              # 8 contraction tiles
CH = 1                   # AllGather chunks (each NBLK/CH dest blocks of h)

_cache = {}


def _preprocess(x, edge_index, W, b):
    x = np.asarray(x, dtype=np.float32)
    ei = np.asarray(edge_index)
    W = np.asarray(W, dtype=np.float32)
    b = np.asarray(b, dtype=np.float32)

    self_idx = np.arange(N, dtype=np.int64)
    row = np.concatenate([ei[0].astype(np.int64), self_idx])
    col = np.concatenate([ei[1].astype(np.int64), self_idx])

    deg = np.bincount(row, minlength=N).astype(np.float32)
    dis = deg ** -0.5  # deg >= 1 (self loops)

    # gathered-table index of each source column: the AllGather runs in CH
    # chunks of RCH rows per rank, so chunk ch of the table holds
    # [rank0 rows [ch*RCH,(ch+1)*RCH), rank1 rows ..., ...].
    CHe = min(CH, NBLK)
    RCH = NPCP // CHe
    csrc = (col // NPC).astype(np.int64)
    l = (col % NPC).astype(np.int64)
    chk = l // RCH
    colp = (chk * (NC * RCH) + csrc * RCH + (l - chk * RCH)).astype(np.int32)

    core_of = (row // NPC).astype(np.int32)
    rl = (row - core_of.astype(np.int64) * NPC).astype(np.int32)  # local dest

    per_cb_cols = [[None] * NBLK for _ in range(NC)]
    per_cb_dl = [[None] * NBLK for _ in range(NC)]
    for c in range(NC):
        m = core_of == c
        rl_c = rl[m]
        cp_c = colp[m]
        order = np.argsort(rl_c, kind="stable")
        rl_c = rl_c[order]
        cp_c = cp_c[order]
        blk = rl_c // P
        bounds = np.searchsorted(blk, np.arange(NBLK + 1))
        for bk in range(NBLK):
            s, e = bounds[bk], bounds[bk + 1]
            per_cb_cols[c][bk] = cp_c[s:e]
            per_cb_dl[c][bk] = rl_c[s:e] - bk * P

    # shared tile schedule: T_b = max over cores of ceil(edges/128)
    T_bs = []
    for bk in range(NBLK):
        mx = max(len(per_cb_cols[c][bk]) for c in range(NC))
        T_bs.append(max(1, -(-mx // P)))
    NT = sum(T_bs)

    # per-core padded arrays: dl [NT,128] f32; gather idx int16 packed
    # [16, T_b*8] per block (idx j at [j%16, j//16]), blocks concatenated
    # along columns, then replicated across the 8 Q7 16-partition stripes.
    dl = np.full((NC, NT, P), -1.0, dtype=np.float32)
    idx16 = np.zeros((NC, 16, NT * 8), dtype=np.int16)
    t0 = 0
    c0 = 0
    for bk in range(NBLK):
        ncols = T_bs[bk] * 8
        for c in range(NC):
            cc = per_cb_cols[c][bk]
            dd = per_cb_dl[c][bk]
            n = len(cc)
            flat_d = dl[c, t0 : t0 + T_bs[bk]].reshape(-1)
            flat_d[:n] = dd.astype(np.float32)
            buf = np.zeros(T_bs[bk] * P, dtype=np.int16)
            buf[:n] = cc.astype(np.int16)
            idx16[c, :, c0 : c0 + ncols] = buf.reshape(ncols, 16).T
        t0 += T_bs[bk]
        c0 += ncols

    WTb = np.ascontiguousarray(W.T).astype(ml_dtypes.bfloat16)  # [d, o]
    bvec = b.reshape(1, D)

    in_maps = []
    for c in range(NC):
        rows = slice(c * NPC, (c + 1) * NPC)
        dis_c = dis[rows]
        xs = x[rows] * dis_c[:, None]
        xT = np.zeros((D, NPCP), dtype=ml_dtypes.bfloat16)
        xT[:, :NPC] = xs.T.astype(ml_dtypes.bfloat16)
        disv = np.zeros((1, NPCP), dtype=np.float32)
        disv[0, :NPC] = dis_c
        disT = np.ascontiguousarray(disv.reshape(NBLK, P).T)  # [128, NBLK]
        in_maps.append(
            {
                "xT": xT,
                "WT": WTb,
                "bvec": bvec,
                "disv": disv,
                "disT": disT,
                "idx16": np.ascontiguousarray(np.tile(idx16[c], (8, 1))),
                "dl": np.ascontiguousarray(dl[c].T),  # [128, NT]
            }
        )
    return tuple(T_bs), NT, in_maps


def _build(T_bs, NT):
    f32 = mybir.dt.float32
    bf16 = mybir.dt.bfloat16
    i16 = mybir.dt.int16
    i32 = mybir.dt.int32
    CHe = min(CH, NBLK)
    RCH = NPCP // CHe
    JCH = NBLK // CHe  # h blocks per AG chunk

    nc = bacc.Bacc("TRN2", target_bir_lowering=False, debug=False, num_devices=NC)
    xT = nc.dram_tensor("xT", [D, NPCP], bf16, kind="ExternalInput").ap()
    WT = nc.dram_tensor("WT", [D, D], bf16, kind="ExternalInput").ap()
    bvec = nc.dram_tensor("bvec", [1, D], f32, kind="ExternalInput").ap()
    disv = nc.dram_tensor("disv", [1, NPCP], f32, kind="ExternalInput").ap()
    disT = nc.dram_tensor("disT", [P, NBLK], f32, kind="ExternalInput").ap()
    idx16 = nc.dram_tensor("idx16", [P, NT * 8], i16, kind="ExternalInput").ap()
    dl = nc.dram_tensor("dl", [P, NT], f32, kind="ExternalInput").ap()
    yout = nc.dram_tensor("yout", [NPCP, D], f32, kind="ExternalOutput").ap()

    with tile.TileContext(nc) as tc:
        with tc.tile_pool(name="dram", bufs=1, space="DRAM") as dram, \
             tc.tile_pool(name="const", bufs=1) as const:
            h_ch = [
                dram.tile([RCH, D], bf16, name=f"h_ch{ch}") for ch in range(CHe)
            ]
            hg = dram.tile([NC * NPCP, D], bf16, addr_space="Shared")

            # constants in SBUF
            wt_sb = const.tile([P, KT * D], bf16, name="wt_sb")
            for k in range(KT):
                nc.sync.dma_start(
                    wt_sb[:, k * D : (k + 1) * D], WT[k * P : (k + 1) * P, :]
                )
            disT_sb = const.tile([P, NBLK], f32, name="disT_sb")
            nc.sync.dma_start(disT_sb[:], disT[:])
            ix_sb = const.tile([P, NT * 8], i16, name="ix_sb")
            nc.sync.dma_start(ix_sb[:], idx16[:])
            dl_sb = const.tile([P, NT], f32, name="dl_sb")
            nc.sync.dma_start(dl_sb[:], dl[:])
            TMAX = max(T_bs)
            iota_rep = const.tile([P, TMAX * P], f32, name="iota_rep")
            with tc.tile_pool(name="tmpi", bufs=1) as tmpp:
                iota_i = tmpp.tile([P, TMAX * P], i32, name="iota_i")
                nc.gpsimd.iota(
                    iota_i[:], pattern=[[0, TMAX], [1, P]], channel_multiplier=0
                )
                nc.vector.tensor_copy(iota_rep[:], iota_i[:])

            # ---------------- h phase (+ chunked AllGather) ----------------
            with tc.tile_pool(name="xk", bufs=1) as xkp, \
                 tc.tile_pool(name="hps", bufs=2, space="PSUM") as hps, \
                 tc.tile_pool(name="hout", bufs=3) as houtp:
                bv_sb = xkp.tile([1, D], f32, name="bv_sb")
                nc.sync.dma_start(bv_sb[:], bvec[:])
                disv_sb = xkp.tile([1, NPCP], f32, name="disv_sb")
                nc.sync.dma_start(disv_sb[:], disv[:])
                xk_sb = xkp.tile([P, KT * NPCP], bf16, name="xk_sb")
                for k in range(KT):
                    nc.sync.dma_start(
                        xk_sb[:, k * NPCP : (k + 1) * NPCP],
                        xT[k * P : (k + 1) * P, :],
                    )
                chunks = [slice(s, min(s + 512, D)) for s in range(0, D, 512)]
                for j in range(NBLK):
                    ps = hps.tile([P, D], f32)
                    for cs in chunks:
                        nc.tensor.matmul(
                            ps[:, cs],
                            lhsT=disv_sb[:, j * P : (j + 1) * P],
                            rhs=bv_sb[:, cs],
                            start=True,
                            stop=False,
                        )
                    for k in range(KT):
                        lhsT = xk_sb[:, k * NPCP + j * P : k * NPCP + (j + 1) * P]
                        for cs in chunks:
                            nc.tensor.matmul(
                                ps[:, cs],
                                lhsT=lhsT,
                                rhs=wt_sb[:, k * D + cs.start : k * D + cs.stop],
                                start=False,
                                stop=(k == KT - 1),
                            )
                    hsb = houtp.tile([P, D], bf16)
                    nc.vector.tensor_copy(hsb[:], ps[:])
                    ch = j // JCH
                    jo = j - ch * JCH
                    nc.sync.dma_start(h_ch[ch][jo * P : (jo + 1) * P, :], hsb[:])
                    if jo == JCH - 1:
                        nc.gpsimd.collective_compute(
                            "AllGather",
                            mybir.AluOpType.bypass,
                            replica_groups=[list(range(NC))],
                            ins=[h_ch[ch][:]],
                            outs=[hg[ch * NC * RCH : (ch + 1) * NC * RCH, :]],
                        )

            # ---------------- aggregation phase ----------------
            with tc.tile_pool(name="gath", bufs=2) as gp, \
                 tc.tile_pool(name="sel", bufs=8) as selp, \
                 tc.tile_pool(name="aps", bufs=2, space="PSUM") as aps, \
                 tc.tile_pool(name="aout", bufs=3) as aoutp:
                t = 0
                c0 = 0
                GSUB = 8  # <=1024 idxs per dma_gather (single-packet limit)
                for bk in range(NBLK):
                    Tb = T_bs[bk]
                    g = gp.tile([P, Tb, D], bf16, tag="g")
                    for s0 in range(0, Tb, GSUB):
                        sn = min(GSUB, Tb - s0)
                        nc.gpsimd.dma_gather(
                            g[:, s0 : s0 + sn, :],
                            hg[:],
                            ix_sb[:, c0 + s0 * 8 : c0 + (s0 + sn) * 8],
                            sn * P,
                            sn * P,
                            D,
                        )
                    selb = selp.tile([P, Tb, P], bf16, tag="selb")
                    dlb = (
                        dl_sb[:, t : t + Tb]
                        .rearrange("p (t o) -> p t o", o=1)
                        .to_broadcast([P, Tb, P])
                    )
                    nc.vector.tensor_tensor(
                        out=selb[:],
                        in0=iota_rep[:, : Tb * P].rearrange(
                            "p (t o) -> p t o", o=P
                        ),
                        in1=dlb,
                        op=mybir.AluOpType.is_equal,
                    )
                    ps = aps.tile([P, D], f32)
                    for i in range(Tb):
                        for cs in [slice(s, min(s + 512, D)) for s in range(0, D, 512)]:
                            nc.tensor.matmul(
                                ps[:, cs],
                                lhsT=selb[:, i, :],
                                rhs=g[:, i, cs],
                                start=(i == 0),
                                stop=(i == Tb - 1),
                            )
                        t += 1
                    ob = aoutp.tile([P, D], f32)
                    nc.vector.tensor_scalar(
                        out=ob[:],
                        in0=ps[:],
                        scalar1=disT_sb[:, bk : bk + 1],
                        scalar2=None,
                        op0=mybir.AluOpType.mult,
                    )
                    nc.sync.dma_start(yout[bk * P : (bk + 1) * P, :], ob[:])
                    c0 += Tb * 8

    nc.compile()
    return nc


def kernel(x, edge_index, W, b):
    T_bs, NT, in_maps = _preprocess(x, edge_index, W, b)
    key = (T_bs, NT)
    if key not in _cache:
        _cache[key] = _build(T_bs, NT)
    nc = _cache[key]
    res = bass_utils.run_bass_kernel_spmd(nc, in_maps, core_ids=list(range(NC)))
    out = np.empty((N, D), dtype=np.float32)
    for c in range(NC):
        out[c * NPC : (c + 1) * NPC] = res.results[c]["yout"][:NPC]
    return out
